# revision 18
# baseline (speedup 1.0000x reference)
"""Adaptive margin loss kernel for 8 TRN2 NeuronCores.

loss = mean((pos-lan)^2) + LAMDA * mean(relu(MARGIN - d2))
  d2[b,c] = mean_d (pos[b,d] - neg[b,c,d])^2

Strategy: data-parallel over batch (32 b per core). feat_neg is transposed
host-side to (B, DP=128, C) (d padded with zeros to 128 so DMA sprays all
16 SDMA engines) with d on SBUF partitions; then (neg - pos)^2 is computed
by ScalarE (func=Square, bias=-pos[b] per-partition) for part of the c
range and by VectorE (tensor_scalar add + square) for the rest; the sum
over d is a TensorE matmul with a ones vector (lhsT=sq chunk (128,128),
rhs=ones -> psum (128,1), c lands on partitions), and the relu + global
accumulation is one ScalarE pass at the end. Each core returns raw partial
sums; the host combines them.
"""

import numpy as np

B, C, D = 256, 4096, 100
DP = 128  # d padded to full partition count
N_CORES = 8
B_LOC = B // N_CORES  # 32
MARGIN = 0.1
LAMDA = 1.0

# 128-wide c-chunk split between ScalarE (direct (neg-pos)^2 via Square+bias)
# and VectorE (expansion: neg^2 via one tensor_mul; cross/P2 terms via extra
# PE matmuls against m2p = [-2*pos; P2])
ACT_CHUNKS = 16
DVE_CHUNKS = 16
NEG_BUFS = 12
NEG_DTYPE = "fp8"  # "bf16" or "fp8" storage for feat_neg

_cached = {}


def _build_bass():
    import concourse.bacc as bacc
    import concourse.tile as tile
    from concourse import mybir

    bf16 = mybir.dt.bfloat16
    f32 = mybir.dt.float32
    neg_dt = bf16 if NEG_DTYPE == "bf16" else mybir.dt.float8e4

    CHUNKS = C // 128  # 32 psum columns per b
    assert ACT_CHUNKS + DVE_CHUNKS == CHUNKS
    C_ACT = ACT_CHUNKS * 128
    C_DVE = DVE_CHUNKS * 128

    nc = bacc.Bacc(
        "TRN2", target_bir_lowering=False, debug=False, num_devices=N_CORES
    )
    neg = nc.declare_dram_parameter("neg", [B_LOC, DP, C], neg_dt, isOutput=False)
    # pl = hstack(-pos.T, pos.T, lan.T) padded to DP rows: one DMA
    pl = nc.declare_dram_parameter("pl", [DP, 3 * B_LOC], f32, isOutput=False)
    # m2p[:100, b] = -2*pos[b] (fp8, must match neg dtype for the PE)
    m2p = nc.declare_dram_parameter("m2p", [DP, B_LOC], neg_dt, isOutput=False)
    # p2row[0, b] = P2[b] = sum_d pos[b,d]^2 (exact f32 term)
    p2row = nc.declare_dram_parameter("p2row", [1, B_LOC], f32, isOutput=False)
    out = nc.declare_dram_parameter("out", [1, 2], f32, isOutput=True)

    with tile.TileContext(nc) as tc:
        with (
            tc.tile_pool(name="big", bufs=NEG_BUFS) as bigp,
            tc.tile_pool(name="sq", bufs=2) as sqp,
            tc.tile_pool(name="small", bufs=1) as small,
            tc.tile_pool(name="psum", bufs=2, space="PSUM") as psump,
        ):
            pl_sb = small.tile([DP, 3 * B_LOC], f32)
            nc.sync.dma_start(out=pl_sb[:], in_=pl[:])
            m2p_sb = small.tile([DP, B_LOC], neg_dt)
            nc.sync.dma_start(out=m2p_sb[:], in_=m2p[:])
            p2_sb = small.tile([1, B_LOC], f32)
            nc.sync.dma_start(out=p2_sb[:], in_=p2row[:])
            ones1 = small.tile([1, 128], f32)
            nc.vector.memset(ones1[:], 1.0)
            # broadcast P2 across partitions once: (128, B_LOC)
            p2ps = psump.tile([128, B_LOC], f32, tag="p2ps")
            nc.tensor.matmul(
                p2ps[:], lhsT=ones1[:], rhs=p2_sb[:], start=True, stop=True
            )
            p2b_sb = small.tile([128, B_LOC], f32)
            nc.vector.tensor_copy(out=p2b_sb[:], in_=p2ps[:])
            posn_sb = pl_sb[:, 0:B_LOC]
            post_sb = pl_sb[:, B_LOC : 2 * B_LOC]
            lant_sb = pl_sb[:, 2 * B_LOC : 3 * B_LOC]

            ones_bf = small.tile([DP, 1], bf16)
            nc.vector.memset(ones_bf[:], 1.0)
            ones128 = small.tile([128, 1], f32)
            nc.vector.memset(ones128[:], 1.0)
            margin_sb = small.tile([128, 1], f32)
            nc.vector.memset(margin_sb[:], MARGIN)

            # per-(b,c) sum-of-squares collected here (f32), 32 cols per b
            coll = small.tile([128, B_LOC * CHUNKS], f32)

            for b in range(B_LOC):
                neg_t = bigp.tile([DP, C], neg_dt)
                nc.sync.dma_start(out=neg_t[:], in_=neg[b])

                sq_a = sqp.tile([DP, C_ACT], bf16, tag="sq_a")
                nc.scalar.activation(
                    out=sq_a[:],
                    in_=neg_t[:, 0:C_ACT],
                    func=mybir.ActivationFunctionType.Square,
                    bias=posn_sb[:, b : b + 1],
                    scale=1.0,
                )
                sq_d = sqp.tile([DP, C_DVE], bf16, tag="sq_d")
                nc.vector.tensor_mul(
                    out=sq_d[:],
                    in0=neg_t[:, C_ACT:],
                    in1=neg_t[:, C_ACT:],
                )

                ps = psump.tile([128, CHUNKS], f32)
                for j in range(ACT_CHUNKS):
                    nc.tensor.matmul(
                        ps[:, j : j + 1],
                        lhsT=sq_a[:, 128 * j : 128 * (j + 1)],
                        rhs=ones_bf[:],
                        start=True,
                        stop=True,
                    )
                for jj in range(DVE_CHUNKS):
                    j = ACT_CHUNKS + jj
                    # sum_d neg^2 (rows >= D contribute 0 via ones mask)
                    nc.tensor.matmul(
                        ps[:, j : j + 1],
                        lhsT=sq_d[:, 128 * jj : 128 * (jj + 1)],
                        rhs=ones_bf[:],
                        start=True,
                        stop=False,
                    )
                    # + sum_d neg * (-2 pos)
                    nc.tensor.matmul(
                        ps[:, j : j + 1],
                        lhsT=neg_t[:, 128 * j : 128 * (j + 1)],
                        rhs=m2p_sb[:, b : b + 1],
                        start=False,
                        stop=True,
                    )
                nc.vector.tensor_copy(
                    out=coll[:, b * CHUNKS : b * CHUNKS + ACT_CHUNKS],
                    in_=ps[:, 0:ACT_CHUNKS],
                )
                # DVE columns need the exact +P2[b] term
                nc.vector.tensor_scalar_add(
                    out=coll[:, b * CHUNKS + ACT_CHUNKS : (b + 1) * CHUNKS],
                    in0=ps[:, ACT_CHUNKS:],
                    scalar1=p2b_sb[:, b : b + 1],
                )

            # relu(margin - x/D) for all (b,c), accumulated per partition
            relu_trash = small.tile([128, B_LOC * CHUNKS], f32)
            r128 = small.tile([128, 1], f32)
            nc.scalar.activation(
                out=relu_trash[:],
                in_=coll[:],
                func=mybir.ActivationFunctionType.Relu,
                scale=-1.0 / D,
                bias=margin_sb[:],
                accum_out=r128[:],
            )

            # loss1 partial: sum over (b_local, d) of (pos - lan)^2 in f32
            # (padded rows are zero on both sides -> contribute 0)
            diff1 = small.tile([DP, B_LOC], f32)
            nc.vector.tensor_sub(out=diff1[:], in0=post_sb, in1=lant_sb)
            st_trash = small.tile([DP, B_LOC], f32)
            l1acc = small.tile([DP, 1], f32)
            nc.vector.scalar_tensor_tensor(
                out=st_trash[:],
                in0=diff1[:],
                scalar=0.0,
                in1=diff1[:],
                op0=mybir.AluOpType.add,
                op1=mybir.AluOpType.mult,
                accum_out=l1acc[:],
            )

            # partition reductions -> scalars, via ones matmuls
            fin = psump.tile([1, 2], f32, tag="fin")
            nc.tensor.matmul(
                fin[:, 0:1], lhsT=r128[:], rhs=ones128[:], start=True, stop=True
            )
            nc.tensor.matmul(
                fin[:, 1:2], lhsT=l1acc[:], rhs=ones128[:], start=True, stop=True
            )
            out_sb = small.tile([1, 2], f32)
            nc.vector.tensor_copy(out=out_sb[:], in_=fin[:])
            nc.sync.dma_start(out=out[:], in_=out_sb[:])

    return nc


def _prep_inputs(feat_pos, feat_neg, feat_lan):
    import ml_dtypes

    feat_pos = np.asarray(feat_pos, dtype=np.float32)
    feat_neg = np.asarray(feat_neg, dtype=np.float32)
    feat_lan = np.asarray(feat_lan, dtype=np.float32)

    # (B, C, D) -> (B, DP, C) with d zero-padded to DP
    neg_np_dt = ml_dtypes.bfloat16 if NEG_DTYPE == "bf16" else ml_dtypes.float8_e4m3
    negT = np.zeros((B, DP, C), dtype=neg_np_dt)
    negT[:, :D, :] = feat_neg.transpose(0, 2, 1).astype(neg_np_dt)


    in_maps = []
    for i in range(N_CORES):
        sl = slice(i * B_LOC, (i + 1) * B_LOC)
        pl = np.zeros((DP, 3 * B_LOC), dtype=np.float32)
        pl[:D, 0:B_LOC] = -feat_pos[sl].T
        pl[:D, B_LOC : 2 * B_LOC] = feat_pos[sl].T
        pl[:D, 2 * B_LOC : 3 * B_LOC] = feat_lan[sl].T
        m2p = np.zeros((DP, B_LOC), dtype=neg_np_dt)
        m2p[:D, :] = (-2.0 * feat_pos[sl].T).astype(neg_np_dt)
        p2 = (feat_pos[sl].astype(np.float64) ** 2).sum(axis=1).astype(np.float32)
        in_maps.append(
            {"neg": negT[sl], "pl": pl, "m2p": m2p, "p2row": p2.reshape(1, -1)}
        )
    return in_maps


def run(feat_pos, feat_neg, feat_lan, trace=False):
    from concourse.bass_utils import run_bass_kernel_spmd

    key = (ACT_CHUNKS, DVE_CHUNKS, NEG_BUFS, NEG_DTYPE, "v4")
    if key not in _cached:
        nc = _build_bass()
        nc.finalize()
        _cached[key] = nc
    nc = _cached[key]

    in_maps = _prep_inputs(feat_pos, feat_neg, feat_lan)
    res = run_bass_kernel_spmd(
        nc, in_maps, core_ids=list(range(N_CORES)), trace=trace
    )
    outs = [r["out"] for r in res.results]
    loss2_sum = float(sum(float(o[0, 0]) for o in outs))
    loss1_sum = float(sum(float(o[0, 1]) for o in outs))
    loss = loss1_sum / (B * D) + LAMDA * loss2_sum / (B * C)
    return np.float32(loss), res


def kernel(feat_pos, feat_neg, feat_lan):
    loss, _ = run(feat_pos, feat_neg, feat_lan, trace=False)
    return loss


# revision 19
# speedup vs baseline: 1.0698x; 1.0698x over previous
"""Adaptive margin loss kernel for 8 TRN2 NeuronCores.

loss = mean((pos-lan)^2) + LAMDA * mean(relu(MARGIN - d2))
  d2[b,c] = mean_d (pos[b,d] - neg[b,c,d])^2

Strategy: data-parallel over batch (32 b per core). feat_neg is transposed
host-side to (B, DP=128, C) (d padded with zeros to 128 so DMA sprays all
16 SDMA engines) with d on SBUF partitions; then (neg - pos)^2 is computed
by ScalarE (func=Square, bias=-pos[b] per-partition) for part of the c
range and by VectorE (tensor_scalar add + square) for the rest; the sum
over d is a TensorE matmul with a ones vector (lhsT=sq chunk (128,128),
rhs=ones -> psum (128,1), c lands on partitions), and the relu + global
accumulation is one ScalarE pass at the end. Each core returns raw partial
sums; the host combines them.
"""

import numpy as np

B, C, D = 256, 4096, 100
DP = 128  # d padded to full partition count
N_CORES = 8
B_LOC = B // N_CORES  # 32
MARGIN = 0.1
LAMDA = 1.0

# 128-wide c-chunk split between ScalarE (direct (neg-pos)^2 via Square+bias)
# and VectorE (expansion: neg^2 via one tensor_mul; cross/P2 terms via extra
# PE matmuls against m2p = [-2*pos; P2])
ACT_CHUNKS = 18
DVE_CHUNKS = 14
NEG_BUFS = 16
NEG_DTYPE = "fp8"  # "bf16" or "fp8" storage for feat_neg

_cached = {}


def _build_bass():
    import concourse.bacc as bacc
    import concourse.tile as tile
    from concourse import mybir

    bf16 = mybir.dt.bfloat16
    f32 = mybir.dt.float32
    neg_dt = bf16 if NEG_DTYPE == "bf16" else mybir.dt.float8e4

    CHUNKS = C // 128  # 32 psum columns per b
    assert ACT_CHUNKS + DVE_CHUNKS == CHUNKS
    C_ACT = ACT_CHUNKS * 128
    C_DVE = DVE_CHUNKS * 128

    nc = bacc.Bacc(
        "TRN2", target_bir_lowering=False, debug=False, num_devices=N_CORES
    )
    neg = nc.declare_dram_parameter("neg", [B_LOC, DP, C], neg_dt, isOutput=False)
    # pl = hstack(-pos.T, pos.T, lan.T) padded to DP rows: one DMA
    pl = nc.declare_dram_parameter("pl", [DP, 3 * B_LOC], f32, isOutput=False)
    # m2p[:100, b] = -2*pos[b] (fp8, must match neg dtype for the PE)
    m2p = nc.declare_dram_parameter("m2p", [DP, B_LOC], neg_dt, isOutput=False)
    # p2row[0, b] = P2[b] = sum_d pos[b,d]^2 (exact f32 term)
    p2row = nc.declare_dram_parameter("p2row", [1, B_LOC], f32, isOutput=False)
    out = nc.declare_dram_parameter("out", [1, 2], f32, isOutput=True)

    with tile.TileContext(nc) as tc:
        with (
            tc.tile_pool(name="big", bufs=NEG_BUFS) as bigp,
            tc.tile_pool(name="sq", bufs=2) as sqp,
            tc.tile_pool(name="small", bufs=1) as small,
            tc.tile_pool(name="psum", bufs=2, space="PSUM") as psump,
        ):
            pl_sb = small.tile([DP, 3 * B_LOC], f32)
            nc.sync.dma_start(out=pl_sb[:], in_=pl[:])
            m2p_sb = small.tile([DP, B_LOC], neg_dt)
            nc.sync.dma_start(out=m2p_sb[:], in_=m2p[:])
            p2_sb = small.tile([1, B_LOC], f32)
            nc.sync.dma_start(out=p2_sb[:], in_=p2row[:])
            ones1 = small.tile([1, 128], f32)
            nc.vector.memset(ones1[:], 1.0)
            # broadcast P2 across partitions once: (128, B_LOC)
            p2ps = psump.tile([128, B_LOC], f32, tag="p2ps")
            nc.tensor.matmul(
                p2ps[:], lhsT=ones1[:], rhs=p2_sb[:], start=True, stop=True
            )
            p2b_sb = small.tile([128, B_LOC], f32)
            nc.vector.tensor_copy(out=p2b_sb[:], in_=p2ps[:])
            # warm up the ACT Square table set while DMA ramps
            warm = small.tile([1, 1], f32)
            nc.scalar.activation(
                out=warm[:], in_=ones1[:, 0:1],
                func=mybir.ActivationFunctionType.Square,
            )
            posn_sb = pl_sb[:, 0:B_LOC]
            post_sb = pl_sb[:, B_LOC : 2 * B_LOC]
            lant_sb = pl_sb[:, 2 * B_LOC : 3 * B_LOC]

            ones_bf = small.tile([DP, 1], bf16)
            nc.vector.memset(ones_bf[:], 1.0)
            ones128 = small.tile([128, 1], f32)
            nc.vector.memset(ones128[:], 1.0)
            margin_sb = small.tile([128, 1], f32)
            nc.vector.memset(margin_sb[:], MARGIN)

            # per-(b,c) sum-of-squares collected here (f32), 32 cols per b
            coll = small.tile([128, B_LOC * CHUNKS], f32)

            for b in range(B_LOC):
                neg_t = bigp.tile([DP, C], neg_dt)
                nc.sync.dma_start(out=neg_t[:], in_=neg[b])

                sq_a = sqp.tile([DP, C_ACT], bf16, tag="sq_a")
                nc.scalar.activation(
                    out=sq_a[:],
                    in_=neg_t[:, 0:C_ACT],
                    func=mybir.ActivationFunctionType.Square,
                    bias=posn_sb[:, b : b + 1],
                    scale=1.0,
                )
                sq_d = sqp.tile([DP, C_DVE], bf16, tag="sq_d")
                nc.vector.tensor_mul(
                    out=sq_d[:],
                    in0=neg_t[:, C_ACT:],
                    in1=neg_t[:, C_ACT:],
                )

                ps = psump.tile([128, CHUNKS], f32)
                for j in range(ACT_CHUNKS):
                    nc.tensor.matmul(
                        ps[:, j : j + 1],
                        lhsT=sq_a[:, 128 * j : 128 * (j + 1)],
                        rhs=ones_bf[:],
                        start=True,
                        stop=True,
                    )
                for jj in range(DVE_CHUNKS):
                    j = ACT_CHUNKS + jj
                    # sum_d neg^2 (rows >= D contribute 0 via ones mask)
                    nc.tensor.matmul(
                        ps[:, j : j + 1],
                        lhsT=sq_d[:, 128 * jj : 128 * (jj + 1)],
                        rhs=ones_bf[:],
                        start=True,
                        stop=False,
                    )
                    # + sum_d neg * (-2 pos)
                    nc.tensor.matmul(
                        ps[:, j : j + 1],
                        lhsT=neg_t[:, 128 * j : 128 * (j + 1)],
                        rhs=m2p_sb[:, b : b + 1],
                        start=False,
                        stop=True,
                    )
                nc.vector.tensor_copy(
                    out=coll[:, b * CHUNKS : b * CHUNKS + ACT_CHUNKS],
                    in_=ps[:, 0:ACT_CHUNKS],
                )
                # DVE columns need the exact +P2[b] term
                nc.vector.tensor_scalar_add(
                    out=coll[:, b * CHUNKS + ACT_CHUNKS : (b + 1) * CHUNKS],
                    in0=ps[:, ACT_CHUNKS:],
                    scalar1=p2b_sb[:, b : b + 1],
                )

            # relu(margin - x/D) for all (b,c), accumulated per partition
            relu_trash = small.tile([128, B_LOC * CHUNKS], f32)
            r128 = small.tile([128, 1], f32)
            nc.scalar.activation(
                out=relu_trash[:],
                in_=coll[:],
                func=mybir.ActivationFunctionType.Relu,
                scale=-1.0 / D,
                bias=margin_sb[:],
                accum_out=r128[:],
            )

            # loss1 partial: sum over (b_local, d) of (pos - lan)^2 in f32
            # (padded rows are zero on both sides -> contribute 0)
            diff1 = small.tile([DP, B_LOC], f32)
            nc.vector.tensor_sub(out=diff1[:], in0=post_sb, in1=lant_sb)
            st_trash = small.tile([DP, B_LOC], f32)
            l1acc = small.tile([DP, 1], f32)
            nc.vector.scalar_tensor_tensor(
                out=st_trash[:],
                in0=diff1[:],
                scalar=0.0,
                in1=diff1[:],
                op0=mybir.AluOpType.add,
                op1=mybir.AluOpType.mult,
                accum_out=l1acc[:],
            )

            # partition reductions -> scalars, via ones matmuls
            fin = psump.tile([1, 2], f32, tag="fin")
            nc.tensor.matmul(
                fin[:, 0:1], lhsT=r128[:], rhs=ones128[:], start=True, stop=True
            )
            nc.tensor.matmul(
                fin[:, 1:2], lhsT=l1acc[:], rhs=ones128[:], start=True, stop=True
            )
            out_sb = small.tile([1, 2], f32)
            nc.vector.tensor_copy(out=out_sb[:], in_=fin[:])
            nc.sync.dma_start(out=out[:], in_=out_sb[:])

    return nc


def _prep_inputs(feat_pos, feat_neg, feat_lan):
    import ml_dtypes

    feat_pos = np.asarray(feat_pos, dtype=np.float32)
    feat_neg = np.asarray(feat_neg, dtype=np.float32)
    feat_lan = np.asarray(feat_lan, dtype=np.float32)

    # (B, C, D) -> (B, DP, C) with d zero-padded to DP
    neg_np_dt = ml_dtypes.bfloat16 if NEG_DTYPE == "bf16" else ml_dtypes.float8_e4m3
    negT = np.zeros((B, DP, C), dtype=neg_np_dt)
    negT[:, :D, :] = feat_neg.transpose(0, 2, 1).astype(neg_np_dt)


    in_maps = []
    for i in range(N_CORES):
        sl = slice(i * B_LOC, (i + 1) * B_LOC)
        pl = np.zeros((DP, 3 * B_LOC), dtype=np.float32)
        pl[:D, 0:B_LOC] = -feat_pos[sl].T
        pl[:D, B_LOC : 2 * B_LOC] = feat_pos[sl].T
        pl[:D, 2 * B_LOC : 3 * B_LOC] = feat_lan[sl].T
        m2p = np.zeros((DP, B_LOC), dtype=neg_np_dt)
        m2p[:D, :] = (-2.0 * feat_pos[sl].T).astype(neg_np_dt)
        p2 = (feat_pos[sl].astype(np.float64) ** 2).sum(axis=1).astype(np.float32)
        in_maps.append(
            {"neg": negT[sl], "pl": pl, "m2p": m2p, "p2row": p2.reshape(1, -1)}
        )
    return in_maps


def run(feat_pos, feat_neg, feat_lan, trace=False):
    from concourse.bass_utils import run_bass_kernel_spmd

    key = (ACT_CHUNKS, DVE_CHUNKS, NEG_BUFS, NEG_DTYPE, "v4")
    if key not in _cached:
        nc = _build_bass()
        nc.finalize()
        _cached[key] = nc
    nc = _cached[key]

    in_maps = _prep_inputs(feat_pos, feat_neg, feat_lan)
    res = run_bass_kernel_spmd(
        nc, in_maps, core_ids=list(range(N_CORES)), trace=trace
    )
    outs = [r["out"] for r in res.results]
    loss2_sum = float(sum(float(o[0, 0]) for o in outs))
    loss1_sum = float(sum(float(o[0, 1]) for o in outs))
    loss = loss1_sum / (B * D) + LAMDA * loss2_sum / (B * C)
    return np.float32(loss), res


def kernel(feat_pos, feat_neg, feat_lan):
    loss, _ = run(feat_pos, feat_neg, feat_lan, trace=False)
    return loss
